# revision 1
# baseline (speedup 1.0000x reference)
"""DeepseekMoE kernel for 8 Trainium2 NeuronCores.

Strategy (expert-parallel + data-parallel shared experts):
  - Host computes the router (gate matmul, softmax, top-2) in numpy and
    gathers each expert's tokens (classic MoE dispatch, done host-side as
    part of sharding).
  - Core c runs routed expert c's FFN over its gathered tokens (padded to
    a common Cpad so all 8 cores run the same SPMD program), scaling the
    output by the combine weights on-device (DVE).
  - Shared experts' weights are replicated; each core runs them over a
    distinct 512-token slice of the batch (data-parallel).
  - All matmuls run in fp16 (1 cycle/row on the PE, same rate as bf16
    but with 10-bit mantissa -> ~8x less rounding error; fp32 is 4x
    slower) with fp32 PSUM accumulation; GELU (exact/erf) on the ACT
    engine reading PSUM directly.
  - Layout is fully transposed (features on partitions, tokens on the
    free dim) so the two FFN matmuls chain with no on-chip transposes.
    Host pre-packs every operand into [128, *] row-major blocks so each
    DMA is a contiguous >=512 KB transfer (HWDGE generation overhead is
    ~625 ns/DMA, so small DMAs cap effective HBM bandwidth).
  - The f-loop is software-pipelined (lookahead 2) across chunks and
    phases so the PE never stalls on ACT; output DMAs ride the SWDGE
    (gpsimd) path so they are not head-of-line blocked behind the
    input preload on the HWDGE queues.
  - Host scatters per-expert outputs back (each token appears in exactly
    K=2 experts) and adds the (zero, but handled exactly) output biases.
"""

import numpy as np
import ml_dtypes

import concourse.bass as bass
import concourse.tile as tile
import concourse.mybir as mybir
from concourse import bacc
from concourse.bass_utils import run_bass_kernel_spmd

B, S, D, F, E, NS, K = 2, 2048, 512, 2048, 8, 2, 2
T = B * S
N_CORES = 8
TS = T // N_CORES          # shared-expert tokens per core
FS = NS * F                # concatenated shared FFN width
CHUNK = 512                # token chunk (= max fp32 PSUM bank free dim)
KD = D // 128              # 4  k-tiles over D
FR = F // 128              # 16 f-tiles routed
FShared = FS // 128        # 32 f-tiles shared
DD = D // 128              # 4  output d-tiles
WG = 4                     # f-tiles per w-DMA group (512 KB transfers)

BF16 = mybir.dt.float16
F32 = mybir.dt.float32
np_bf16 = np.float16

_GELU = mybir.ActivationFunctionType.Gelu

_cache: dict = {}


def _routed_sizes(cpad):
    """Token-chunk sizes for the routed phase: a small first chunk (fast PE
    start — less DMA to wait for), 512s in the middle, and a smallish final
    chunk (short drain tail). No chunk below 256 — small-N matmuls go
    LDWEIGHTS-bound on real hardware."""
    if cpad <= CHUNK:
        return [cpad]
    head = cpad - 768
    if 256 <= head <= CHUNK:                # the realistic range
        return [head, CHUNK, 256]
    if cpad < 1024:
        mid = cpad - 512
        return [256] + ([mid] if mid else []) + [256]
    sizes, rem = [256], cpad - 768          # reserve two 256 tail chunks
    while rem > CHUNK:
        take = CHUNK if rem - CHUNK >= 256 else rem - 256
        sizes.append(take)
        rem -= take
    sizes.append(rem)
    return sizes + [256, 256]


def _shared_sizes(ts):
    """Shared-expert chunk sizes; ends on a 256 chunk for a short tail."""
    return [ts] if ts <= 256 else [ts - 256, 256]


def _chunk_offsets(total, sizes=None):
    """(start, size) pairs; default uniform CHUNK split."""
    if sizes is None:
        sizes = [min(CHUNK, total - c0) for c0 in range(0, total, CHUNK)]
    out, c0 = [], 0
    for s in sizes:
        out.append((c0, s))
        c0 += s
    return out


def _build(cpad: int):
    nc = bacc.Bacc("TRN2", debug=False)

    xg = nc.dram_tensor("xg", [128, KD * cpad], BF16, kind="ExternalInput")
    cwb = nc.dram_tensor("cwb", [128, cpad], F32, kind="ExternalInput")
    rw1t = nc.dram_tensor("rw1t", [128, KD * F], BF16, kind="ExternalInput")
    rw2t = nc.dram_tensor("rw2t", [128, FR * D], BF16, kind="ExternalInput")
    rb1 = nc.dram_tensor("rb1", [128, FR], F32, kind="ExternalInput")
    xs = nc.dram_tensor("xs", [128, KD * TS], BF16, kind="ExternalInput")
    sw1t = nc.dram_tensor("sw1t", [128, KD * FS], BF16, kind="ExternalInput")
    sw2t = nc.dram_tensor("sw2t", [128, FShared * D], BF16, kind="ExternalInput")
    sb1 = nc.dram_tensor("sb1", [128, FShared], F32, kind="ExternalInput")
    yr = nc.dram_tensor("yr", [D, cpad], F32, kind="ExternalOutput")
    ys = nc.dram_tensor("ys", [D, TS], BF16, kind="ExternalOutput")

    with tile.TileContext(nc) as tc:
        with (
            tc.tile_pool(name="wts", bufs=1) as wts,
            tc.tile_pool(name="acts", bufs=1) as acts,
            tc.tile_pool(name="hp", bufs=4) as hp,
            tc.tile_pool(name="op", bufs=3) as op,
            tc.tile_pool(name="ps1", bufs=4, space="PSUM") as ps1,
            tc.tile_pool(name="ps2", bufs=1, space="PSUM") as ps2,
        ):
            # ---- t=0 warmup while the first DMAs are in flight: trigger the
            # GELU ACT-table load now (it costs ~1.3 us on first use), and run
            # dummy matmuls so the PE p-state/HAM is at full clock when the
            # first real matmul issues ----
            warm = wts.tile([128, 512], BF16, name="warm_in")
            nc.vector.memset(warm[:], 0.0)
            wb = wts.tile([128, 1], F32, name="warm_b")
            nc.vector.memset(wb[:], 0.0)
            wh = hp.tile([128, 512], BF16, name="wh")
            nc.scalar.activation(wh[:], warm[:, 0:512], _GELU, bias=wb[:])
            wp = ps1.tile([128, 512], F32, tag="p1", name="warmp")
            for _ in range(6):
                nc.tensor.matmul(wp[:], warm[:, 0:128], warm[:], start=True, stop=True)

            # ---- resident SBUF images of all inputs ----
            xg_sb = acts.tile([128, KD * cpad], BF16, name="xg_sb")
            rw1_sb = wts.tile([128, KD * F], BF16, name="rw1_sb")
            rw2_sb = wts.tile([128, FR * D], BF16, name="rw2_sb")
            rb1_sb = wts.tile([128, FR], F32, name="rb1_sb")
            cw_sb = acts.tile([128, cpad], F32, name="cw_sb")
            xs_sb = acts.tile([128, KD * TS], BF16, name="xs_sb")
            sw1_sb = wts.tile([128, KD * FS], BF16, name="sw1_sb")
            sw2_sb = wts.tile([128, FShared * D], BF16, name="sw2_sb")
            sb1_sb = wts.tile([128, FShared], F32, name="sb1_sb")

            def col_dma(dst, src, lo, hi):
                nc.sync.dma_start(dst[:, lo:hi], src.ap()[:, lo:hi])

            def w1_group_dma(dst, src, f_lo, f_hi):
                # f-columns [f_lo*128, f_hi*128) for every k-block
                d4 = dst.rearrange("p (k f) -> p k f", k=KD)
                s4 = src.ap().rearrange("p (k f) -> p k f", k=KD)
                nc.sync.dma_start(d4[:, :, f_lo * 128:f_hi * 128],
                                  s4[:, :, f_lo * 128:f_hi * 128])

            # consumption-ordered preload (HWDGE)
            chunks_r = _chunk_offsets(cpad, _routed_sizes(cpad))
            c0, cs = chunks_r[0]
            xoff = [0]
            for _, s in chunks_r:
                xoff.append(xoff[-1] + KD * s)
            # chunk-0 tokens ride SWDGE so their descriptor generation runs in
            # parallel with rw1's HWDGE generation (shorter startup chain)
            nc.gpsimd.dma_start(xg_sb[:, 0:xoff[1]], xg.ap()[:, 0:xoff[1]])
            w1_group_dma(rw1_sb, rw1t, 0, 2)                    # rw1 f0..f1
            nc.sync.dma_start(rb1_sb[:], rb1.ap())
            col_dma(rw2_sb, rw2t, 0, WG * D)                    # rw2 f0..f3
            w1_group_dma(rw1_sb, rw1t, 2, 4)
            for g in range(1, FR // WG):
                w1_group_dma(rw1_sb, rw1t, g * WG, (g + 1) * WG)
                col_dma(rw2_sb, rw2t, g * WG * D, (g + 1) * WG * D)
            col_dma(xg_sb, xg, xoff[1], xoff[-1])               # remaining tokens
            nc.sync.dma_start(cw_sb[:], cwb.ap())
            nc.sync.dma_start(xs_sb[:], xs.ap())
            nc.sync.dma_start(sb1_sb[:], sb1.ap())
            for g in range(FShared // (2 * WG)):                # 1 MB transfers
                w1_group_dma(sw1_sb, sw1t, g * 2 * WG, (g + 1) * 2 * WG)
                col_dma(sw2_sb, sw2t, g * 2 * WG * D, (g + 1) * 2 * WG * D)

            # ---- chunk descriptors: small routed chunk first (fast start),
            # shared phase last, ending on a small chunk (short tail) ----
            def r_chunk(i, c0, cs):
                return dict(
                    cs=cs, c0=c0, nf=FR, cw=True, y=yr, b1=rb1_sb,
                    x=lambda k, o=xoff[i], cs=cs: xg_sb[:, o + k * cs:o + (k + 1) * cs],
                    w1=lambda k, f: rw1_sb[:, k * F + f * 128:k * F + (f + 1) * 128],
                    w2=lambda f, d: rw2_sb[:, f * D + d * 128:f * D + (d + 1) * 128],
                )

            def s_chunk(i, c0, cs):
                return dict(
                    cs=cs, c0=c0, nf=FShared, cw=False, y=ys, b1=sb1_sb,
                    x=lambda k, i=i, cs=cs: xs_sb[:, soff[i] + k * cs:soff[i] + (k + 1) * cs],
                    w1=lambda k, f: sw1_sb[:, k * FS + f * 128:k * FS + (f + 1) * 128],
                    w2=lambda f, d: sw2_sb[:, f * D + d * 128:f * D + (d + 1) * 128],
                )

            chunks_s = _chunk_offsets(TS, _shared_sizes(TS))
            soff = [0]
            for _, s in chunks_s:
                soff.append(soff[-1] + KD * s)
            routed = [r_chunk(i, c0, cs) for i, (c0, cs) in enumerate(chunks_r)]
            shared = [s_chunk(i, c0, cs) for i, (c0, cs) in enumerate(chunks_s)]
            chunks = routed + shared
            steps = [(ch, f) for ch in chunks for f in range(ch["nf"])]

            # ---- software-pipelined emission: PE issues the f-tile's
            # first-layer matmuls LOOKAHEAD steps ahead of the second-layer
            # matmuls that consume the GELU output ----
            LOOKAHEAD = 2
            h_tiles: dict = {}
            po_tiles: dict = {}
            for i in range(len(steps) + LOOKAHEAD):
                if i < len(steps):
                    ch, f = steps[i]
                    cs = ch["cs"]
                    p1 = ps1.tile([128, cs], F32, name="p1")
                    for k in range(KD):
                        nc.tensor.matmul(
                            p1[:], ch["w1"](k, f), ch["x"](k),
                            start=(k == 0), stop=(k == KD - 1),
                        )
                    h = hp.tile([128, cs], BF16, name="h")
                    nc.scalar.activation(h[:], p1[:], _GELU, bias=ch["b1"][:, f:f + 1])
                    h_tiles[i] = h
                j = i - LOOKAHEAD
                if j >= 0:
                    ch, f = steps[j]
                    cs, c0 = ch["cs"], ch["c0"]
                    if f == 0:
                        po_tiles[id(ch)] = [
                            ps2.tile([128, cs], F32, tag=f"o{d}", name=f"po{d}")
                            for d in range(DD)
                        ]
                    po = po_tiles[id(ch)]
                    h = h_tiles.pop(j)
                    for d in range(DD):
                        nc.tensor.matmul(
                            po[d][:], ch["w2"](f, d), h[:],
                            start=(f == 0), stop=(f == ch["nf"] - 1),
                        )
                    if f == ch["nf"] - 1:
                        o = op.tile([128, DD * cs], F32 if ch["cw"] else BF16,
                                    name="o")
                        last = ch is chunks[-1]
                        for d in range(DD):
                            if ch["cw"]:
                                nc.vector.tensor_mul(
                                    o[:, d * cs:(d + 1) * cs], po[d][:],
                                    cw_sb[:, c0:c0 + cs])
                            elif last and d >= 2:
                                # tail chunk: split evacuation across ACT and
                                # DVE so the final drain starts sooner
                                nc.scalar.copy(o[:, d * cs:(d + 1) * cs], po[d][:])
                            else:
                                nc.vector.tensor_copy(
                                    o[:, d * cs:(d + 1) * cs], po[d][:])
                        # one wide DMA per chunk on the SWDGE path: separate
                        # FIFO from the input preload (no head-of-line block),
                        # and one generation overhead instead of four. The
                        # final chunk rides HWDGE (lower latency; preload is
                        # long finished) to shorten the kernel tail.
                        ydst = ch["y"].ap().rearrange(
                            "(dd p) c -> p dd c", p=128)[:, :, c0:c0 + cs]
                        ysrc = o.rearrange("p (dd c) -> p dd c", dd=DD)
                        if last:
                            nc.sync.dma_start(ydst, ysrc)
                        else:
                            nc.gpsimd.dma_start(ydst, ysrc)
                        del po_tiles[id(ch)]

    nc.compile()
    return nc


def _pack_k_blocks(a2d):
    """[K*128, N] -> [128, K*N] with k-blocks along the free dim."""
    k = a2d.shape[0] // 128
    return np.ascontiguousarray(
        a2d.reshape(k, 128, -1).transpose(1, 0, 2).reshape(128, -1))


def _pack_chunked(xT, total, sizes=None):
    """[D, total] -> [128, KD*total] grouped chunk-major: for each chunk c,
    the KD k-blocks of that chunk's columns are laid out consecutively."""
    parts = []
    for c0, cs in _chunk_offsets(total, sizes):
        blk = xT[:, c0:c0 + cs]                      # [D, cs]
        parts.append(blk.reshape(KD, 128, cs).transpose(1, 0, 2).reshape(128, -1))
    return np.ascontiguousarray(np.concatenate(parts, axis=1))


def kernel(x, gate_w, gate_b, sw1, sb1, sw2, sb2, rw1, rb1, rw2, rb2):
    x = np.asarray(x, np.float32)
    gate_w = np.asarray(gate_w, np.float32)
    gate_b = np.asarray(gate_b, np.float32)
    sw1 = np.asarray(sw1, np.float32)
    sb1 = np.asarray(sb1, np.float32)
    sw2 = np.asarray(sw2, np.float32)
    sb2 = np.asarray(sb2, np.float32)
    rw1 = np.asarray(rw1, np.float32)
    rb1 = np.asarray(rb1, np.float32)
    rw2 = np.asarray(rw2, np.float32)
    rb2 = np.asarray(rb2, np.float32)

    t = x.reshape(T, D)

    # ---- router on host (part of the dispatch/sharding step) ----
    logits = t @ gate_w.T + gate_b
    m = logits.max(axis=1, keepdims=True)
    ex = np.exp(logits - m)
    probs = ex / ex.sum(axis=1, keepdims=True)
    top_i = np.argpartition(-probs, K - 1, axis=1)[:, :K]          # [T, K]

    sel = np.zeros((T, E), bool)
    sel[np.arange(T)[:, None], top_i] = True
    idxs = [np.nonzero(sel[:, e])[0] for e in range(E)]
    counts = np.array([len(i) for i in idxs])
    cpad = max(CHUNK, int(-(-counts.max() // 4) * 4))

    if cpad not in _cache:
        _cache[cpad] = _build(cpad)
    nc = _cache[cpad]

    # ---- shared-expert weights, concatenated over NS and packed ----
    sw1t = _pack_k_blocks(sw1.reshape(FS, D).T.astype(np_bf16))
    sw2t = _pack_k_blocks(sw2.transpose(0, 2, 1).reshape(FS, D).astype(np_bf16))
    sb1c = np.ascontiguousarray(sb1.reshape(FShared, 128).T)

    in_maps = []
    for c in range(N_CORES):
        idx = idxs[c]
        ce = len(idx)
        xgT = np.zeros((D, cpad), np_bf16)
        xgT[:, :ce] = t[idx].T.astype(np_bf16)
        cwb = np.zeros((128, cpad), np.float32)
        cwb[:, :ce] = probs[idx, c][None, :]
        in_maps.append({
            "xg": _pack_chunked(xgT, cpad, _routed_sizes(cpad)),
            "cwb": cwb,
            "rw1t": _pack_k_blocks(rw1[c].T.astype(np_bf16)),
            "rw2t": _pack_k_blocks(rw2[c].T.astype(np_bf16)),
            "rb1": np.ascontiguousarray(rb1[c].reshape(FR, 128).T),
            "xs": _pack_chunked(
                np.ascontiguousarray(t[c * TS:(c + 1) * TS].T.astype(np_bf16)),
                TS, _shared_sizes(TS)),
            "sw1t": sw1t,
            "sw2t": sw2t,
            "sb1": sb1c,
        })

    res = run_bass_kernel_spmd(nc, in_maps, core_ids=list(range(N_CORES)))

    # ---- combine on host ----
    out = np.empty((T, D), np.float32)
    for c in range(N_CORES):
        out[c * TS:(c + 1) * TS] = res.results[c]["ys"].T.astype(np.float32)
    for c in range(N_CORES):
        idx = idxs[c]
        out[idx] += res.results[c]["yr"][:, :len(idx)].T

    # output biases (zero in the spec, handled exactly anyway)
    if sb2.any() or rb2.any():
        cw = np.zeros((T, E), np.float32)
        np.add.at(cw, (np.arange(T)[:, None], top_i),
                  np.take_along_axis(probs, top_i, axis=1))
        out += sb2.sum(axis=0)[None, :] + cw @ rb2

    return out.reshape(B, S, D)



# revision 7
# speedup vs baseline: 1.5777x; 1.5777x over previous
"""DeepseekMoE kernel for 8 Trainium2 NeuronCores.

Strategy (expert-parallel routed + data-parallel shared, fp8 PE path):
  - Host computes the router (gate matmul, softmax, top-2) in numpy and
    gathers each expert's tokens (classic MoE dispatch, done host-side as
    part of sharding).  Core c runs routed expert c's FFN over its
    gathered tokens (padded to a common cpad so all 8 cores run the same
    SPMD program); shared experts are replicated and each core runs them
    over a distinct 512-token slice of the batch.
  - All matmuls run in fp8-E4M3 with the PE's DoubleRow perf mode (two
    128-row k-tiles contracted per instruction at 0.5 cycles/row -> 4x
    the fp16 rate).  Weights are scaled by 64 before quantization so
    their 0.02-scale values leave the E4M3 subnormal range; the 1/64
    descale folds into the GELU's input scale (layer 1), the combine
    weights (routed layer 2) or the output copy (shared layer 2).
  - The routed path (only ~4.5% of the output energy after the top-2
    combine weights) tolerates plain fp8 quantization of x, h and both
    weight matrices (~1.1e-2 end-to-end rel err vs the 2e-2 budget).
  - The shared path (~97% of output energy) cannot; each shared operand
    is a same-scale fp8 PAIR (hi = fp8(v), lo = fp8(v - hi)) and each
    matmul contracts 3 of the 4 cross terms -- hi@a + lo@a + hi@b --
    which restores ~fp16 accuracy at 0.75x the fp16 PE cost.  x's pair
    is built on the host; h's pair is built on-device (ACT writes
    gelu->fp16, DVE casts the fp8 hi image, Pool subtracts for lo).
  - Shared phase runs FIRST: its 8 MB of weight DMA streams in
    consumption order behind a 41 us compute phase, and the (smaller)
    routed weights stream during the shared tail, so the DMA engines are
    never on the critical path.
  - Layout is fully transposed (features on partitions, tokens on the
    free dim) so the two FFN layers chain with no on-chip transposes;
    every operand is host-packed so each DMA is a contiguous >=2KB-per-
    partition transfer.  Outputs ride the SWDGE (gpsimd) path except the
    final chunk (HWDGE, shorter tail).
  - Host scatters per-expert outputs back (each token appears in K=2
    experts) and adds the (zero, but handled exactly) output biases.
"""

import numpy as np
import ml_dtypes

import concourse.bass as bass
import concourse.tile as tile
import concourse.mybir as mybir
from concourse import bacc
from concourse.bass_utils import run_bass_kernel_spmd

B, S, D, F, E, NS, K = 2, 2048, 512, 2048, 8, 2, 2
T = B * S
N_CORES = 8
TS = T // N_CORES          # shared-expert tokens per core
FS = NS * F                # concatenated shared FFN width (4096)
CHUNK = 512                # token chunk (= max fp32 PSUM bank free dim)
KD = D // 128              # 4  k-tiles over D
FR = F // 128              # 16 f-tiles routed
FPR = FR // 2              # 8  routed f-pairs
FSH = FS // 128            # 32 f-tiles shared
FPS = FSH // 2             # 16 shared f-pairs
DD = D // 128              # 4  output d-tiles
SC = 64.0                  # fp8 weight pre-scale (lifts 0.02-scale
                           # weights out of E4M3's subnormal range)

F8 = mybir.dt.float8e4
F16 = mybir.dt.float16
BF16 = mybir.dt.bfloat16
F32 = mybir.dt.float32
np_f8 = ml_dtypes.float8_e4m3
np_bf16 = ml_dtypes.bfloat16

DR = mybir.MatmulPerfMode.DoubleRow
_GELU = mybir.ActivationFunctionType.Gelu

_cache: dict = {}


def _routed_sizes(cpad):
    """Token-chunk sizes for the routed phase: a mid-size first chunk,
    512s in the middle, and a small final chunk (short drain tail)."""
    if cpad <= CHUNK:
        return [cpad]
    head = cpad - 768
    if 256 <= head <= CHUNK:
        return [head, CHUNK, 256]
    if cpad < 1024:
        mid = cpad - 512
        return [256] + ([mid] if mid else []) + [256]
    sizes, rem = [256], cpad - 768
    while rem > CHUNK:
        take = CHUNK if rem - CHUNK >= 256 else rem - 256
        sizes.append(take)
        rem -= take
    sizes.append(rem)
    return sizes + [256, 256]


def _chunk_offsets(total, sizes):
    out, c0 = [], 0
    for s in sizes:
        out.append((c0, s))
        c0 += s
    return out


def _build(cpad: int):
    nc = bacc.Bacc("TRN2", debug=False)

    # -- routed inputs (expert c on core c), all fp8 pre-scaled by 64 --
    xg = nc.dram_tensor("xg", [128, KD * cpad], F8, kind="ExternalInput")
    cwb = nc.dram_tensor("cwb", [128, cpad], F32, kind="ExternalInput")
    # rw1: col = f*(KD*128) + k*128 ; stationary [k-rows, f-cols] blocks
    rw1t = nc.dram_tensor("rw1t", [128, FR * KD * 128], F8, kind="ExternalInput")
    # rw2: col = fp*(DD*256) + d*256 + two*128
    rw2t = nc.dram_tensor("rw2t", [128, FPR * DD * 256], F8, kind="ExternalInput")
    rb1 = nc.dram_tensor("rb1", [128, FR], F32, kind="ExternalInput")
    # -- shared inputs, fp8 hi/lo pairs --
    # xs: col = img*(KD*TS) + k*TS   (img 0 = hi, 1 = lo)
    xs = nc.dram_tensor("xs", [128, 2 * KD * TS], F8, kind="ExternalInput")
    # sw1: col = f*(2*KD*128) + img*(KD*128) + k*128
    sw1t = nc.dram_tensor("sw1t", [128, FSH * 2 * KD * 128], F8, kind="ExternalInput")
    # sw2: col = fp*(DD*512) + d*512 + img*256 + two*128
    sw2t = nc.dram_tensor("sw2t", [128, FPS * DD * 512], F8, kind="ExternalInput")
    sb1 = nc.dram_tensor("sb1", [128, FSH], F32, kind="ExternalInput")
    yr = nc.dram_tensor("yr", [D, cpad], BF16, kind="ExternalOutput")
    ys = nc.dram_tensor("ys", [D, TS], BF16, kind="ExternalOutput")

    chunks_r = _chunk_offsets(cpad, _routed_sizes(cpad))
    xoff = [0]
    for _, s in chunks_r:
        xoff.append(xoff[-1] + KD * s)

    with tile.TileContext(nc) as tc:
        with (
            tc.tile_pool(name="wts", bufs=1) as wts,
            tc.tile_pool(name="acts", bufs=1) as acts,
            tc.tile_pool(name="hp", bufs=3) as hp,
            tc.tile_pool(name="op", bufs=3) as op,
            tc.tile_pool(name="ps1", bufs=4, space="PSUM") as ps1,
            tc.tile_pool(name="ps2", bufs=1, space="PSUM") as ps2,
        ):
            # ---- PE p-state warmup while the first DMAs are in flight ----
            warm = wts.tile([128, 512], F16, name="warm_in")
            nc.vector.memset(warm[:], 0.0)
            wp = ps1.tile([128, 512], F32, tag="p1", name="warmp")
            for _ in range(6):
                nc.tensor.matmul(wp[:], warm[:, 0:128], warm[:], start=True, stop=True)

            # ---- resident SBUF images ----
            xs_sb = acts.tile([128, 2 * KD * TS], F8, name="xs_sb")
            sw1_sb = wts.tile([128, FSH * 2 * KD * 128], F8, name="sw1_sb")
            sw2_sb = wts.tile([128, FPS * DD * 512], F8, name="sw2_sb")
            sb1_sb = wts.tile([128, FSH], F32, name="sb1_sb")
            xg_sb = acts.tile([128, KD * cpad], F8, name="xg_sb")
            rw1_sb = wts.tile([128, FR * KD * 128], F8, name="rw1_sb")
            rw2_sb = wts.tile([128, FPR * DD * 256], F8, name="rw2_sb")
            rb1_sb = wts.tile([128, FR], F32, name="rb1_sb")
            cw_sb = acts.tile([128, cpad], F32, name="cw_sb")

            def col_dma(dst, src, lo, hi):
                nc.sync.dma_start(dst[:, lo:hi], src.ap()[:, lo:hi])

            # ---- consumption-ordered preload (HWDGE), shared first ----
            col_dma(xs_sb, xs, 0, 2 * KD * TS)
            nc.sync.dma_start(sb1_sb[:], sb1.ap())
            # shared weight stream: sw1 in groups of 4 f-tiles (4KB/part),
            # sw2 in groups of 2 f-pairs (4KB/part), interleaved so each
            # arrives ~2 f-steps before its consumption.
            SW1G, SW2G = 4 * 2 * KD * 128, 2 * DD * 512
            order = [("w1", 0), ("w1", 1), ("w1", 2), ("w2", 0),
                     ("w1", 3), ("w2", 1), ("w1", 4), ("w2", 2),
                     ("w1", 5), ("w2", 3), ("w1", 6), ("w2", 4),
                     ("w1", 7), ("w2", 5), ("w2", 6), ("w2", 7)]
            for kind, g in order:
                if kind == "w1":
                    col_dma(sw1_sb, sw1t, g * SW1G, (g + 1) * SW1G)
                else:
                    col_dma(sw2_sb, sw2t, g * SW2G, (g + 1) * SW2G)
            nc.sync.dma_start(rb1_sb[:], rb1.ap())
            # routed stream (consumed after the shared phase)
            col_dma(xg_sb, xg, 0, xoff[1])
            RW1G, RW2G = 4 * KD * 128, 2 * DD * 256
            for g in range(4):
                col_dma(rw1_sb, rw1t, g * RW1G, (g + 1) * RW1G)
                col_dma(rw2_sb, rw2t, g * RW2G, (g + 1) * RW2G)
            col_dma(xg_sb, xg, xoff[1], xoff[-1])
            nc.sync.dma_start(cw_sb[:], cwb.ap())

            def pair3(ap2):
                """[128, 2*n] -> [128, 2, n] (DoubleRow two-k-tile view)."""
                return ap2.rearrange("p (a c) -> p a c", a=2)

            # ================= phase 1: shared experts =================
            # one 512-token chunk; L1 3-term fp8 pairs; h split on device.
            po_s = [ps2.tile([128, TS], F32, tag=f"o{d}", name=f"po{d}")
                    for d in range(DD)]
            sh_state: dict = {}

            def sh_l1(f):
                p1 = ps1.tile([128, TS], F32, name="p1")
                wbase = f * (2 * KD * 128)
                a01 = pair3(sw1_sb[:, wbase + 0:wbase + 256])
                a23 = pair3(sw1_sb[:, wbase + 256:wbase + 512])
                b01 = pair3(sw1_sb[:, wbase + 512:wbase + 768])
                b23 = pair3(sw1_sb[:, wbase + 768:wbase + 1024])
                xh01 = pair3(xs_sb[:, 0:2 * TS])
                xh23 = pair3(xs_sb[:, 2 * TS:4 * TS])
                xl01 = pair3(xs_sb[:, 4 * TS:6 * TS])
                xl23 = pair3(xs_sb[:, 6 * TS:8 * TS])
                mms = [(a01, xh01), (a01, xl01), (a23, xh23), (a23, xl23),
                       (b01, xh01), (b23, xh23)]
                for i, (w, x) in enumerate(mms):
                    nc.tensor.matmul(p1[:], w, x, start=(i == 0),
                                     stop=(i == len(mms) - 1), perf_mode=DR)
                # gelu (descaled by 1/64) -> fp16, then split into the
                # fp8 hi/lo pair for layer 2's 3-term contraction
                if f % 2 == 0:
                    sh_state["h16"] = hp.tile([128, 2 * TS], F16, name="h16")
                    sh_state["hh"] = hp.tile([128, 2 * TS], F8, name="hh")
                    sh_state["hl"] = hp.tile([128, 2 * TS], F8, name="hl")
                h16, hh, hl = sh_state["h16"], sh_state["hh"], sh_state["hl"]
                sl = slice((f % 2) * TS, (f % 2 + 1) * TS)
                nc.scalar.activation(h16[:, sl], p1[:], _GELU,
                                     bias=sb1_sb[:, f:f + 1], scale=1.0 / SC)
                nc.vector.tensor_copy(hh[:, sl], h16[:, sl])
                nc.gpsimd.tensor_sub(hl[:, sl], h16[:, sl], hh[:, sl])
                if f % 2 == 1:
                    return (sh_state["hh"], sh_state["hl"])
                return None

            def sh_l2(fp, hpair):
                hh2, hl2 = pair3(hpair[0][:]), pair3(hpair[1][:])
                for d in range(DD):
                    base = fp * (DD * 512) + d * 512
                    a2 = pair3(sw2_sb[:, base:base + 256])
                    b2 = pair3(sw2_sb[:, base + 256:base + 512])
                    out = po_s[d][:]
                    nc.tensor.matmul(out, a2, hh2, start=(fp == 0), stop=False,
                                     perf_mode=DR)
                    nc.tensor.matmul(out, a2, hl2, start=False, stop=False,
                                     perf_mode=DR)
                    nc.tensor.matmul(out, b2, hh2, start=False,
                                     stop=(fp == FPS - 1), perf_mode=DR)

            def sh_drain():
                o = op.tile([128, DD * TS], BF16, name="o_s")
                for d in range(DD):
                    nc.vector.tensor_scalar_mul(
                        o[:, d * TS:(d + 1) * TS], po_s[d][:], 1.0 / SC)
                ydst = ys.ap().rearrange("(dd p) c -> p dd c", p=128)
                nc.gpsimd.dma_start(ydst, o.rearrange("p (dd c) -> p dd c", dd=DD))

            # ================= phase 2: routed expert ==================
            ro_state: dict = {}

            def ro_l1(ci, f, c0, cs):
                p1 = ps1.tile([128, cs], F32, name="p1")
                wbase = f * (KD * 128)
                w01 = pair3(rw1_sb[:, wbase + 0:wbase + 256])
                w23 = pair3(rw1_sb[:, wbase + 256:wbase + 512])
                xb = xoff[ci]
                x01 = pair3(xg_sb[:, xb + 0:xb + 2 * cs])
                x23 = pair3(xg_sb[:, xb + 2 * cs:xb + 4 * cs])
                nc.tensor.matmul(p1[:], w01, x01, start=True, stop=False,
                                 perf_mode=DR)
                nc.tensor.matmul(p1[:], w23, x23, start=False, stop=True,
                                 perf_mode=DR)
                if f % 2 == 0:
                    ro_state["h8"] = hp.tile([128, 2 * cs], F8, name="h8")
                h8 = ro_state["h8"]
                sl = slice((f % 2) * cs, (f % 2 + 1) * cs)
                nc.scalar.activation(h8[:, sl], p1[:], _GELU,
                                     bias=rb1_sb[:, f:f + 1], scale=1.0 / SC)
                if f % 2 == 1:
                    return h8
                return None

            def ro_l2(ci, fp, h8, po_r):
                if fp == 0:
                    cs = chunks_r[ci][1]
                    po_r[ci] = [ps2.tile([128, cs], F32, tag=f"o{d}",
                                         name=f"po{d}") for d in range(DD)]
                po = po_r[ci]
                h2 = pair3(h8[:])
                for d in range(DD):
                    base = fp * (DD * 256) + d * 256
                    w2 = pair3(rw2_sb[:, base:base + 256])
                    nc.tensor.matmul(po[d][:], w2, h2,
                                     start=(fp == 0), stop=(fp == FPR - 1),
                                     perf_mode=DR)

            def ro_drain(ci, po):
                c0, cs = chunks_r[ci]
                o = op.tile([128, DD * cs], BF16, name="o_r")
                for d in range(DD):
                    nc.vector.tensor_mul(o[:, d * cs:(d + 1) * cs], po[d][:],
                                         cw_sb[:, c0:c0 + cs])
                ydst = yr.ap().rearrange("(dd p) c -> p dd c", p=128)[:, :, c0:c0 + cs]
                ysrc = o.rearrange("p (dd c) -> p dd c", dd=DD)
                if ci == len(chunks_r) - 1:
                    nc.sync.dma_start(ydst, ysrc)
                else:
                    nc.gpsimd.dma_start(ydst, ysrc)

            # ---- software-pipelined emission: layer-2 consumption lags
            # layer-1 production so the PE never waits on ACT/DVE/Pool ----
            # step entries: ("s", f) shared, ("r", ci, f) routed
            steps = [("s", f) for f in range(FSH)]
            for ci in range(len(chunks_r)):
                steps += [("r", ci, f) for f in range(FR)]
            LA = {"s": 4, "r": 2}
            pend: list = []   # (due_step, kind, payload)
            po_r: dict = {}

            for i in range(len(steps) + 6):
                if i < len(steps):
                    st = steps[i]
                    if st[0] == "s":
                        hpair = sh_l1(st[1])
                        if hpair is not None:
                            pend.append((i + LA["s"], "s", (st[1] // 2, hpair)))
                        if st[1] == FSH - 1:
                            pend.append((i + LA["s"] + 1, "sd", None))
                    else:
                        _, ci, f = st
                        h8 = ro_l1(ci, f, *chunks_r[ci])
                        if h8 is not None:
                            pend.append((i + LA["r"], "r", (ci, f // 2, h8)))
                        if f == FR - 1:
                            pend.append((i + LA["r"] + 1, "rd", ci))
                # NOTE: head-blocking FIFO -- entries pop strictly in append
                # order, so the shared drain (queued before any routed L2)
                # always frees the o0..o3 PSUM banks before routed reuse.
                while pend and pend[0][0] <= i:
                    _, kind, pl = pend.pop(0)
                    if kind == "s":
                        sh_l2(*pl)
                    elif kind == "sd":
                        sh_drain()
                    elif kind == "r":
                        ci, fp, h8 = pl
                        ro_l2(ci, fp, h8, po_r)
                    elif kind == "rd":
                        ro_drain(pl, po_r.pop(pl))

    nc.compile()
    return nc


def _f8(a):
    return np.asarray(a, np_f8)


def _hilo(a):
    """Same-scale fp8 pair: hi = fp8(a), lo = fp8(a - hi)."""
    hi = _f8(a)
    lo = _f8(np.asarray(a, np.float32) - hi.astype(np.float32))
    return hi, lo


def _pack_sw1(sw1):
    W1 = sw1.reshape(FS, D).astype(np.float32) * SC
    hi, lo = _hilo(W1)
    st = np.stack([hi, lo])                         # [img, FS, D]
    v = st.reshape(2, FSH, 128, KD, 128)            # img f ff k kk
    v = v.transpose(4, 1, 0, 3, 2)                  # kk f img k ff
    return np.ascontiguousarray(v.reshape(128, -1))


def _pack_sw2(sw2):
    W2 = sw2.transpose(0, 2, 1).reshape(FS, D).astype(np.float32) * SC
    hi, lo = _hilo(W2)
    st = np.stack([hi, lo])                         # [img, FS, D]
    v = st.reshape(2, FPS, 2, 128, DD, 128)         # img fp two ff d dd
    v = v.transpose(3, 1, 4, 0, 2, 5)               # ff fp d img two dd
    return np.ascontiguousarray(v.reshape(128, -1))


def _pack_rw1(w):
    A = _f8(w.astype(np.float32) * SC)              # [F, D]
    v = A.reshape(FR, 128, KD, 128)                 # f ff k kk
    v = v.transpose(3, 0, 2, 1)                     # kk f k ff
    return np.ascontiguousarray(v.reshape(128, -1))


def _pack_rw2(w):
    A = _f8(w.T.astype(np.float32) * SC)            # [F, D]
    v = A.reshape(FPR, 2, 128, DD, 128)             # fp two ff d dd
    v = v.transpose(2, 0, 3, 1, 4)                  # ff fp d two dd
    return np.ascontiguousarray(v.reshape(128, -1))


def _pack_xs(xT):
    hi, lo = _hilo(xT)                              # [D, TS] each
    st = np.stack([hi, lo])                         # img D TS
    v = st.reshape(2, KD, 128, TS).transpose(2, 0, 1, 3)   # kk img k c
    return np.ascontiguousarray(v.reshape(128, -1))


def _pack_xg(xT, sizes):
    parts = []
    for c0, cs in _chunk_offsets(xT.shape[1], sizes):
        blk = xT[:, c0:c0 + cs]
        parts.append(blk.reshape(KD, 128, cs).transpose(1, 0, 2).reshape(128, -1))
    return np.ascontiguousarray(np.concatenate(parts, axis=1))


def kernel(x, gate_w, gate_b, sw1, sb1, sw2, sb2, rw1, rb1, rw2, rb2):
    x = np.asarray(x, np.float32)
    gate_w = np.asarray(gate_w, np.float32)
    gate_b = np.asarray(gate_b, np.float32)
    sw1 = np.asarray(sw1, np.float32)
    sb1 = np.asarray(sb1, np.float32)
    sw2 = np.asarray(sw2, np.float32)
    sb2 = np.asarray(sb2, np.float32)
    rw1 = np.asarray(rw1, np.float32)
    rb1 = np.asarray(rb1, np.float32)
    rw2 = np.asarray(rw2, np.float32)
    rb2 = np.asarray(rb2, np.float32)

    t = x.reshape(T, D)

    # ---- router on host (part of the dispatch/sharding step) ----
    logits = t @ gate_w.T + gate_b
    m = logits.max(axis=1, keepdims=True)
    ex = np.exp(logits - m)
    probs = ex / ex.sum(axis=1, keepdims=True)
    top_i = np.argpartition(-probs, K - 1, axis=1)[:, :K]          # [T, K]

    sel = np.zeros((T, E), bool)
    sel[np.arange(T)[:, None], top_i] = True
    idxs = [np.nonzero(sel[:, e])[0] for e in range(E)]
    counts = np.array([len(i) for i in idxs])
    cpad = max(CHUNK, int(-(-counts.max() // 4) * 4))

    if cpad not in _cache:
        _cache[cpad] = _build(cpad)
    nc = _cache[cpad]

    sw1t = _pack_sw1(sw1)
    sw2t = _pack_sw2(sw2)
    sb1c = np.ascontiguousarray(sb1.reshape(FSH, 128).T)
    rsizes = _routed_sizes(cpad)

    in_maps = []
    for c in range(N_CORES):
        idx = idxs[c]
        ce = len(idx)
        xgT = np.zeros((D, cpad), np_f8)
        xgT[:, :ce] = _f8(t[idx].T)
        cwbm = np.zeros((128, cpad), np.float32)
        cwbm[:, :ce] = (probs[idx, c] / SC)[None, :]
        in_maps.append({
            "xg": _pack_xg(xgT, rsizes),
            "cwb": cwbm,
            "rw1t": _pack_rw1(rw1[c]),
            "rw2t": _pack_rw2(rw2[c]),
            "rb1": np.ascontiguousarray(rb1[c].reshape(FR, 128).T),
            "xs": _pack_xs(np.ascontiguousarray(t[c * TS:(c + 1) * TS].T)),
            "sw1t": sw1t,
            "sw2t": sw2t,
            "sb1": sb1c,
        })

    res = run_bass_kernel_spmd(nc, in_maps, core_ids=list(range(N_CORES)))

    # ---- combine on host ----
    out = np.empty((T, D), np.float32)
    for c in range(N_CORES):
        out[c * TS:(c + 1) * TS] = res.results[c]["ys"].T.astype(np.float32)
    for c in range(N_CORES):
        idx = idxs[c]
        out[idx] += res.results[c]["yr"][:, :len(idx)].T.astype(np.float32)

    # output biases (zero in the spec, handled exactly anyway)
    if sb2.any() or rb2.any():
        cw = np.zeros((T, E), np.float32)
        np.add.at(cw, (np.arange(T)[:, None], top_i),
                  np.take_along_axis(probs, top_i, axis=1))
        out += sb2.sum(axis=0)[None, :] + cw @ rb2

    return out.reshape(B, S, D)
